# revision 23
# baseline (speedup 1.0000x reference)
"""Balanced supervised contrastive regression loss on 8 trn2 cores.

Math: rows of `projections` are unit-norm, so rowmax(logits) = 1/T exactly and
E = exp(P@P.T/T - 1/T) + 1e-5. With tw_i = weights[t_i-40], A = E*tw_i*tw_j:
denom[i,j] depends on i only through v = t_i (121 distinct label values), so
the torch-style cubic tensor collapses to label space. Device computes, per
anchor column i (256 per core, anchors data-parallel over 8 cores):
  gacc[u,i]   = sum_k tw_k*1[l_k=u]*et[k,i]     (et = exp((s-1)/T))
  gacc[121,i] = sum_k tw_k*et[k,i]              (denom diagonal row-sum)
  slacc[i]    = sum_k ln(1e5*et[k,i] + 1)       (= sum_k ln E[k,i] + N*ln 1e5)
via PE matmuls (fp8 DoubleRow logits chain, PSUM fp32 accumulate; bf16
reductions) and 2 ACT passes per [128, 1024] quad-chunk. The exp bias is
avoided by computing et' = exp(s/T) and folding e^(-1/T) into the host-side
tw prescale (and the Ln input scale), so activations carry no extra deps.
The +1e-5 floor enters as closed-form host corrections; the one-hot*tw matrix
is built on the idle DVE from labels; host assembles the loss in label space
with prefix-sum gathers (121 x N), never materializing anything cubic.
"""
import os
import numpy as np

N, D, VOCAB, OFF = 2048, 512, 121, 40
TEMP = 0.07
NCORES = 8
R = N // NCORES   # 256 anchor columns per core
KC = N // 128     # 16 chunks of 128 k-rows
CHUNKS = [(0, 2), (2, 4), (6, 4), (10, 4), (14, 2)]  # (kc_start, width) tiles
DC = D // 128     # 4 chunks of the contraction dim
GW = VOCAB + 1    # 122: one-hot*tw columns + tw column
GO = GW + 4       # gout rows: gacc(122) + 4 log-sum partial rows
AW = VOCAB + 2 * KC  # aux input: [iota(121) | (lbl,tw) x 16]
PSCALE = 16.0     # fp8: prescale P into e4m3's sweet spot
EFLOOR = float(np.exp(-1.0 / TEMP))  # folded exp bias

MODE = os.environ.get("KERNEL_MODE", "fp8")  # "fp8" | "bf16"

LAST_EXEC_NS = None
LAST_RESULTS = None


def _build_nc(mode):
    import concourse.bass as bass
    import concourse.mybir as mybir
    from concourse import tile

    import bass_rust as _bass_rust

    f32 = mybir.dt.float32
    bf16 = mybir.dt.bfloat16
    fp8 = mybir.dt.float8e4
    AF = mybir.ActivationFunctionType
    Alu = mybir.AluOpType
    nc = bass.Bass()

    if mode == "fp8":
        # d packed [ds(2), s(2), p(128)]: DoubleRow contracts 256 d-rows/instr
        FLATK = 2 * 2 * 128
        ptb_d = nc.declare_dram_parameter("ptb", [128, KC, FLATK], fp8, isOutput=False)
        ptr_d = nc.declare_dram_parameter("ptr", [128, 2 * 2 * R], fp8, isOutput=False)
        act_scale = 1.0 / (TEMP * PSCALE * PSCALE)
    else:
        FLATK = DC * 128
        ptb_d = nc.declare_dram_parameter("ptb", [128, KC, FLATK], bf16, isOutput=False)
        ptr_d = nc.declare_dram_parameter("ptr", [128, DC * R], bf16, isOutput=False)
        act_scale = 1.0 / TEMP
    aux_d = nc.declare_dram_parameter("aux", [128, AW], f32, isOutput=False)
    gout_d = nc.declare_dram_parameter("gout", [GW, R], f32, isOutput=True)
    slout_d = nc.declare_dram_parameter("slout", [1, 4 * R], f32, isOutput=True)

    pm = mybir.MatmulPerfMode.DoubleRow if mode == "fp8" else None

    with tile.TileContext(nc) as tc:
        with (
            tc.tile_pool(name="sb", bufs=1) as cpool,
            tc.tile_pool(name="ps", bufs=1, space="PSUM") as pspool,
        ):
            wpool, apool = cpool, pspool
            if mode == "fp8":
                ptr_t = cpool.tile([128, 2, 2, R], fp8, tag="ptr")
            else:
                ptr_t = cpool.tile([128, DC, R], bf16, tag="ptr")
            nc.sync.dma_start(ptr_t[:], ptr_d[:])

            # critical-path chunks first on SP (transfer order follows issue
            # order); later chunks stream from the Pool queue; aux last
            dma_eng = [nc.sync, nc.sync, nc.gpsimd, nc.gpsimd, nc.gpsimd]
            ptk = []
            for ci, (s, w) in enumerate(CHUNKS):
                if mode == "fp8":
                    t = cpool.tile([128, w, 2, 2, 128], fp8, tag=f"ptk{ci}")
                else:
                    t = cpool.tile([128, w, DC, 128], bf16, tag=f"ptk{ci}")
                dma_eng[ci].dma_start(t[:], ptb_d[:, s:s + w, :])
                ptk.append(t)
            aux_t = cpool.tile([128, AW], f32, tag="aux")
            nc.sync.dma_start(aux_t[:], aux_d[:])

            ones_t = cpool.tile([128, 1], bf16, tag="ones")
            nc.vector.memset(ones_t[:], 1.0)

            # one-hot*tw weight blocks built on the idle DVE from labels
            owt_t = cpool.tile([128, KC, GW], bf16, tag="owt")
            for kc in range(KC):
                lblap = aux_t[:, VOCAB + 2 * kc:VOCAB + 2 * kc + 1]
                twap = aux_t[:, VOCAB + 2 * kc + 1:VOCAB + 2 * kc + 2]
                nc.vector.tensor_scalar(
                    owt_t[:, kc, 0:VOCAB], aux_t[:, 0:VOCAB],
                    lblap, twap, Alu.is_equal, Alu.mult,
                )
                nc.vector.tensor_copy(owt_t[:, kc, VOCAB:GW], twap)

            gacc = apool.tile([GW, R], f32, tag="gacc")
            slacc = apool.tile([1, 4 * R], f32, tag="slacc")
            # which chunks contribute to each of the two slacc column groups
            gfirst = {0: 0, 1: 1}
            glast = {0: len(CHUNKS) - 1, 1: len(CHUNKS) - 2}

            def gacc_mms(s, w, et):
                for h in range(w):
                    kc = s + h
                    nc.tensor.matmul(gacc[:], owt_t[:, kc, :], et[:, h * R:(h + 1) * R],
                                     start=(kc == 0), stop=(kc == KC - 1))

            ss_t = wpool.tile([1, 4 * R], f32, tag="ss")

            def sl_mms(ci, w, lg):
                for g in range(w // 2):
                    nc.tensor.matmul(slacc[:, g * 2 * R:(g + 1) * 2 * R], ones_t[:],
                                     lg[:, g * 2 * R:(g + 1) * 2 * R],
                                     start=(ci == gfirst[g]), stop=(ci == glast[g]))
                    if g == 1 and ci == glast[1]:
                        # g1 chain complete: evacuate its half early (idle DVE)
                        nc.vector.tensor_copy(ss_t[:, 2 * R:4 * R],
                                              slacc[:, 2 * R:4 * R])

            # two-deep software pipeline: ln(ci-1) queues on ACT after exp(ci)
            # so ACT never stalls on the write-ack of its own just-written et
            p1 = None  # (ci, s, w, et, lt-consumed)
            p2 = None  # (ci, w, lg)
            for ci, (s, w) in enumerate(CHUNKS):
                lt = pspool.tile([128, w * R], f32, tag="lt", padded_shape=[128, 4 * R], bufs=2)
                for h in range(w):
                    if mode == "fp8":
                        for ds in range(2):
                            nc.tensor.matmul(
                                lt[:, h * R:(h + 1) * R],
                                ptk[ci][:, h, ds, :, :],
                                ptr_t[:, ds, :, :],
                                start=(ds == 0), stop=(ds == 1),
                                perf_mode=pm,
                            )
                    else:
                        for dcs in range(DC):
                            nc.tensor.matmul(
                                lt[:, h * R:(h + 1) * R],
                                ptk[ci][:, h, dcs, :],
                                ptr_t[:, dcs, :],
                                start=(dcs == 0), stop=(dcs == DC - 1),
                            )
                et = wpool.tile([128, w * R], bf16, tag="et", padded_shape=[128, 4 * R], bufs=3)
                nc.scalar.activation(et[:], lt[:], AF.Exp, bias=0.0, scale=act_scale)
                if p1 is not None:
                    ci1, s1, w1, et1 = p1
                    gacc_mms(s1, w1, et1)
                    lg1 = wpool.tile([128, w1 * R], bf16, tag="lg", padded_shape=[128, 4 * R], bufs=3)
                    nc.scalar.activation(lg1[:], et1[:], AF.Ln, bias=1.0, scale=1e5 * EFLOOR)
                    if p2 is not None:
                        sl_mms(*p2)
                    p2 = (ci1, w1, lg1)
                p1 = (ci, s, w, et)
            ci1, s1, w1, et1 = p1
            gacc_mms(s1, w1, et1)
            lg1 = wpool.tile([128, w1 * R], bf16, tag="lg", padded_shape=[128, 4 * R], bufs=3)
            nc.scalar.activation(lg1[:], et1[:], AF.Ln, bias=1.0, scale=1e5 * EFLOOR)
            if p2 is not None:
                sl_mms(*p2)
            sl_mms(ci1, w1, lg1)

            gs = wpool.tile([GW, R], f32, tag="gs")
            nc.vector.tensor_copy(gs[:], gacc[:])
            nc.sync.dma_start(gout_d[:], gs[:])
            nc.scalar.copy(ss_t[:, 0:2 * R], slacc[:, 0:2 * R])
            nc.scalar.dma_start(slout_d[:], ss_t[:])
    # hardware allows at most one sync wait per instruction (two on
    # InstEventSemaphore): legalize multi-wait instructions before walrus
    _bass_rust.move_matmul_waits_to_ldweights(nc.m)
    _bass_rust.generate_event_semaphores(nc)
    return nc


def _prep_inputs(P, lbl, twf, mode):
    """Host-side packing of the SPMD input maps (per-core ptr differs)."""
    from concourse.mybir import dt as _dt
    np_bf16 = _dt.np(_dt.bfloat16)

    if mode == "fp8":
        np_fp8 = _dt.np(_dt.float8e4)
        Ps = (P * PSCALE).astype(np_fp8)
        # ptb[p, kc, ds, s, k] = Ps[kc*128 + k, (ds*2+s)*128 + p]
        ptb = np.ascontiguousarray(
            Ps.reshape(KC, 128, 2, 2, 128).transpose(4, 0, 2, 3, 1)
        ).reshape(128, KC, 2 * 2 * 128)
        ptrs = []
        for c in range(NCORES):
            Pc = Ps[c * R:(c + 1) * R]  # [R, 512]
            ptr = np.ascontiguousarray(
                Pc.reshape(R, 2, 2, 128).transpose(3, 1, 2, 0)
            ).reshape(128, 2 * 2 * R)
            ptrs.append(ptr)
    else:
        Pb = P.astype(np_bf16)
        # ptb[p, kc, dc, k] = Pb[kc*128 + k, dc*128 + p]
        ptb = np.ascontiguousarray(
            Pb.reshape(KC, 128, DC, 128).transpose(3, 0, 2, 1)
        ).reshape(128, KC, DC * 128)
        ptrs = []
        for c in range(NCORES):
            Pc = Pb[c * R:(c + 1) * R]
            ptr = np.ascontiguousarray(
                Pc.reshape(R, DC, 128).transpose(2, 1, 0)
            ).reshape(128, DC * R)
            ptrs.append(ptr)

    # aux[p, :] = [iota(121) | lbl_kc, tw_kc*e^(-1/T) for kc in 0..15]  (f32)
    aux = np.zeros((128, AW), np.float32)
    aux[:, :VOCAB] = np.arange(VOCAB, dtype=np.float32)[None, :]
    aux[:, VOCAB::2] = lbl.reshape(KC, 128).T.astype(np.float32)
    aux[:, VOCAB + 1::2] = (twf * EFLOOR).reshape(KC, 128).T

    in_maps = []
    for c in range(NCORES):
        in_maps.append({"ptb": ptb, "ptr": ptrs[c], "aux": aux})
    return in_maps


def _device_run(P, lbl, twf, mode):
    from concourse.bass_utils import run_bass_kernel_spmd

    nc = _build_nc(mode)
    in_maps = _prep_inputs(P, lbl, twf, mode)
    br = run_bass_kernel_spmd(nc, in_maps, list(range(NCORES)))
    global LAST_EXEC_NS, LAST_RESULTS
    LAST_RESULTS = br
    LAST_EXEC_NS = br.exec_time_ns
    res = br.results
    gacc = np.concatenate([np.asarray(r["gout"]) for r in res], 1)   # [122, N]
    sl4 = np.concatenate([np.asarray(r["slout"]).reshape(4, R) for r in res], 1)
    slacc = sl4.sum(0)                                               # [N]
    return gacc.astype(np.float32), slacc.astype(np.float32)


def _host_fallback(P, lbl, twf):
    s = (P.astype(np.float64) @ P.astype(np.float64).T)
    et = np.exp((s - 1.0) / TEMP)
    ohw = np.zeros((N, GW), np.float64)
    ohw[np.arange(N), lbl] = twf
    ohw[:, VOCAB] = twf
    gacc = ohw.T @ et
    slacc = np.log(1e5 * et + 1.0).sum(0)
    return gacc, slacc


def _assemble(gacc, slacc, lbl, tw):
    TWS = tw.sum()
    Q = gacc[:VOCAB].T.astype(np.float64)            # [N,121]: Q[j,u]
    rsE = gacc[VOCAB].astype(np.float64) + 1e-5 * TWS
    sumlogE = slacc.astype(np.float64) - N * np.log(1e5)

    cw = np.bincount(lbl, weights=tw, minlength=VOCAB)
    W = Q + 1e-5 * cw[None, :]
    PS1 = np.concatenate([np.zeros((N, 1)), np.cumsum(W, 1)], 1)  # [N,122]

    vcol = np.arange(VOCAB)[:, None]
    B = np.abs(vcol - lbl[None, :])                  # [121, N]
    lo = np.clip(vcol - B + 1, 0, VOCAB)
    hi1 = np.clip(vcol + B, 0, VOCAB)
    jj = np.arange(N)[None, :]
    inner = PS1[jj, hi1] - PS1[jj, lo]
    inner[B == 0] = 0.0
    Dv = rsE[None, :] - inner                        # [121, N]
    ltw = np.log(tw)
    SLT = ltw.sum()
    LDsum = SLT + np.log(Dv).sum(1)                  # [121]

    rowsumA = tw * rsE
    rowsumLA = sumlogE + N * ltw + SLT
    LAdiag = np.log1p(1e-5) + 2.0 * ltw
    per = (LDsum[lbl] - np.log(rowsumA) - (rowsumLA - LAdiag)) / (N - 1 + 1e-5)
    return per.mean()


def kernel(projections, targets, weights):
    P = np.asarray(projections, np.float32)
    t = np.asarray(targets).astype(np.int64)
    w = np.asarray(weights, np.float64)
    lbl = (t - OFF).astype(np.int64)
    tw = w[lbl]
    twf = tw.astype(np.float32)

    try:
        gacc, slacc = _device_run(P, lbl, twf, MODE)
    except Exception as e:  # pragma: no cover - safety net
        import traceback
        traceback.print_exc()
        print("DEVICE PATH FAILED - host fallback:", e)
        gacc, slacc = _host_fallback(P, lbl, twf)

    return np.float32(_assemble(gacc, slacc, lbl, tw))
